# revision 76
# baseline (speedup 1.0000x reference)
"""Trainium2 Bass kernel for nn_JanusModel (sparse_attention, GQA, two mask groups).

Sharding: core c in [0,8) handles batch b=c//4 and query-row block q0=(c%4)*512.
Each core computes all 16 heads for its 512 query rows -> disjoint output slices,
no collectives. Heavy operands laid out on host (transposes/permutes, exp(mask),
1/sqrt(hd) folded into wq, bf16 casts).

On-device math per core (all bf16 operands, fp32 PSUM):
  qT/kT/v projections, scoresT = K @ qT (row-tiled head pairs, [sk, sq] layout),
  P = exp(scores) * expm (ACT exp + DVE bf16 mul), AV matmuls with a ones-column
  appended to V so row 64 of each AV psum accumulates the softmax denominator,
  reciprocal + DMA broadcast, output projection accumulated over head pairs.

Pipelining: pair 0's whole attention is interleaved into the projection loop so
the ACT engine starts exp work ~10us in; pairs 1-7 run a software-pipelined
PE emission (scores of group g+1 issued before AV of group g); attnT division
for pair j is deferred to pair j+2 so the rowsum DMA roundtrip never stalls PE.
"""

import os
import sys

import numpy as np

for _p in ("/opt/trn_rl_repo",):
    if os.path.isdir(_p) and _p not in sys.path:
        sys.path.insert(0, _p)

import concourse.bass as bass
import concourse.tile as tile
from concourse import bacc, mybir
from concourse.bass_utils import run_bass_kernel_spmd

B, S, D = 2, 2048, 1024
H, KVH, HD = 16, 4, 64
NCORES = 8
SQ = S // 4          # 512 query rows per core
P = 128
NKT = S // P         # 16 key tiles
KC = D // P          # 8 contraction chunks for projections
NG = 8               # 2-tile k groups per half

# Head pairs: (a, b) share a kT tile; a uses kv head 2*(j//4), b uses +1.
PAIRS = [(0, 4), (1, 5), (2, 6), (3, 7), (8, 12), (9, 13), (10, 14), (11, 15)]

f32 = mybir.dt.float32
bf16 = mybir.dt.bfloat16
EXP = mybir.ActivationFunctionType.Exp

_CACHE = {}


def _body(tc, xT, wqT, wkT, wvT, woT, emT, out):
    nc = tc.nc
    rs_dram = nc.dram_tensor("rs_scratch", [8, 2, SQ], bf16).ap()
    xT_r = xT.rearrange("(kc p) (c s) -> c p kc s", p=P, s=SQ)   # [4,128,8,512]
    wqT_r = wqT.rearrange("j (p kc) f -> j p kc f", p=P)         # [8,128,8,128]
    wkT_r = wkT.rearrange("(kc p) f -> p kc f", p=P)             # [128,8,256]
    wvT_r = wvT.rearrange("(kc p) f -> p kc f", p=P)             # [128,8,256]
    woT_r = woT.rearrange("(j p) d -> p j d", p=P)               # [128,8,1024]
    emT_r = emT.rearrange("m (t p) q -> m p t q", p=P)           # [2,128,16,512]
    out_r = out.rearrange("(t p) (n q) -> t n p q", p=P, q=SQ)   # [4,2,128,512]

    persist = tc.alloc_tile_pool(name="persist", bufs=1)
    qT_sb = persist.tile([P, 8, SQ], bf16, name="qT_sb")      # pair j: a 0:64, b 64:128
    kT_sb = persist.tile([P, 2, S], bf16, name="kT_sb")       # jt: kv 2jt 0:64, 2jt+1 64:128
    v_sb = persist.tile([P, NKT, KVH, HD + 1], bf16, name="v_sb")  # col HD = ones
    em_sb = persist.tile([P, 2, NKT, SQ], bf16, name="em_sb")
    attnT_sb = persist.tile([P, 8, SQ], bf16, name="attnT_sb")
    wo_sb = persist.tile([P, 8, D], bf16, name="wo_sb")
    warm = persist.tile([1, 2], bf16, name="warm")

    # ---- all input DMAs, ordered for earliest compute start (one SP queue) ----
    pav = tc.alloc_tile_pool(name="pav", bufs=1, space="PSUM")
    with tc.tile_pool(name="xw", bufs=1) as xw, \
         tc.tile_pool(name="prp", bufs=3) as prp, \
         tc.tile_pool(name="ppp", bufs=6) as ppp, \
         tc.tile_pool(name="avsb", bufs=3) as avsbp, \
         tc.tile_pool(name="small", bufs=2) as small:
        x_sb = xw.tile([P, KC, S], bf16, name="x_sb")
        wq_sb = xw.tile([P, 8, KC, P], bf16, name="wq_sb")
        wk_sb = xw.tile([P, KC, KVH * HD], bf16, name="wk_sb")
        wv_sb = xw.tile([P, KC, KVH * HD], bf16, name="wv_sb")
        nc.sync.dma_start(out=wv_sb, in_=wvT_r)
        nc.sync.dma_start(out=x_sb[:, :, 0:P], in_=xT_r[0, :, :, 0:P])
        nc.sync.dma_start(out=x_sb[:, :, P:SQ], in_=xT_r[0, :, :, P:SQ])
        nc.sync.dma_start(out=wk_sb, in_=wkT_r)
        nc.sync.dma_start(out=wq_sb[:, 0], in_=wqT_r[0])
        nc.sync.dma_start(out=em_sb[:, 0, 0:8, :], in_=emT_r[0, :, 0:8, :])
        nc.sync.dma_start(out=x_sb[:, :, SQ:2 * SQ], in_=xT_r[1])
        nc.sync.dma_start(out=wq_sb[:, 1], in_=wqT_r[1])
        nc.sync.dma_start(out=em_sb[:, 0, 8:16, :], in_=emT_r[0, :, 8:16, :])
        nc.sync.dma_start(out=x_sb[:, :, 2 * SQ:3 * SQ], in_=xT_r[2])
        for j in (2, 3):
            nc.sync.dma_start(out=wq_sb[:, j], in_=wqT_r[j])
        nc.sync.dma_start(out=x_sb[:, :, 3 * SQ:4 * SQ], in_=xT_r[3])
        for j in (4, 5, 6, 7):
            nc.sync.dma_start(out=wq_sb[:, j], in_=wqT_r[j])
        nc.sync.dma_start(out=em_sb[:, 1], in_=emT_r[1])
        nc.sync.dma_start(out=wo_sb, in_=woT_r)

        # ACT exp table warm-up + ones column of v_sb
        nc.vector.memset(warm, 0.0)
        nc.scalar.activation(out=warm, in_=warm, func=EXP)
        nc.vector.memset(v_sb[:, :, :, HD:HD + 1], 1.0)

        # ---------- emission helpers ----------
        def emit_scores(pool, tag, j, t0, ntg, half):
            """ntg score matmuls for k-tiles t0.. -> exp -> mask mul; returns pp."""
            jt = j // 4
            r0, r1 = 64 * half, 64 * half + 64
            sc = pool.tile([P, ntg, SQ], f32, tag=tag, name=f"sc{j}_{t0}_{half}")
            for i in range(ntg):
                t = t0 + i
                nc.tensor.matmul(
                    sc[:, i, :], lhsT=kT_sb[r0:r1, jt, t * P:(t + 1) * P],
                    rhs=qT_sb[r0:r1, j, :], start=True, stop=True)
            pr = prp.tile([P, ntg, SQ], bf16, tag="pr", name=f"pr{j}_{t0}_{half}")
            nc.scalar.activation(out=pr, in_=sc, func=EXP)
            pp = ppp.tile([P, ntg, SQ], bf16, tag="pp", name=f"pp{j}_{t0}_{half}")
            nc.vector.tensor_mul(pp, pr, em_sb[:, j // 4, t0:t0 + ntg, :])
            return pp

        def emit_av(av, j, t0, ntg, half, pp):
            kv = 2 * (j // 4) + half
            for i in range(ntg):
                t = t0 + i
                nc.tensor.matmul(
                    av[0:HD + 1, :], lhsT=v_sb[:, t, kv, :], rhs=pp[:, i, :],
                    start=(t == 0), stop=(t == NKT - 1))

        def emit_rowsum(j, av_a, av_b):
            """Copy AV numerators+rowsums psum->sbuf bf16 (the only readers of
            the av psum banks, so they free ~1.3us after the pair ends), then
            reciprocal from the copy, DMA roundtrip broadcast in bf16. The
            attnT division is deferred to pair j+2. Returns (bc, avsb)."""
            avsb = avsbp.tile([P, 2, SQ], bf16, tag="av", name=f"avsb{j}")
            nc.vector.tensor_copy(out=avsb[0:HD + 1, 0, :], in_=av_a[0:HD + 1, :])
            nc.vector.tensor_copy(out=avsb[0:HD + 1, 1, :], in_=av_b[0:HD + 1, :])
            rr = small.tile([P, 2, SQ], bf16, tag="rr", name=f"rr{j}")
            with nc.allow_low_precision(reason="bf16 softmax denominators"):
                nc.vector.reciprocal(out=rr[HD:HD + 1, 0, :],
                                     in_=avsb[HD:HD + 1, 0, :])
                nc.vector.reciprocal(out=rr[HD:HD + 1, 1, :],
                                     in_=avsb[HD:HD + 1, 1, :])
            nc.sync.dma_start(out=rs_dram[j], in_=rr[HD:HD + 1, :, :])
            bc = small.tile([P, 2, SQ], bf16, tag="bc", name=f"bc{j}")
            for half in range(2):
                row = rs_dram[j, half, :]
                bcast = bass.AP(tensor=row.tensor, offset=row.offset,
                                ap=[[0, 64]] + list(row.ap))
                nc.sync.dma_start(out=bc[0:64, half, :], in_=bcast)
            return bc, avsb

        def emit_attnT(j, pend):
            bc, avsb = pend
            nc.vector.tensor_mul(attnT_sb[0:64, j, :], avsb[0:HD, 0, :],
                                 bc[0:64, 0, :])
            nc.vector.tensor_mul(attnT_sb[64:128, j, :], avsb[0:HD, 1, :],
                                 bc[0:64, 1, :])

        # ---------------- phase A + pairs 0,1 interleaved ----------------
        # Pairs 0 and 1 share one score psum buffer (exps ping-pong through
        # it), keeping ACT busy while the PE runs projection chains. AV
        # consumption lags scores until the v tiles exist; leftover pp tiles
        # are retained in the ppp pool.
        pend = {}
        pp_store = {}
        sc_next = {0: 0, 1: 0}   # next k-TILE (1-tile groups during phase A)
        av_next = {0: 0, 1: 0}
        with tc.tile_pool(name="pps", bufs=2, space="PSUM") as pps, \
             tc.tile_pool(name="psc0", bufs=2, space="PSUM") as psc0, \
             tc.tile_pool(name="pav1", bufs=1, space="PSUM") as pav1:
            avs = {0: (pav.tile([P, SQ], f32, tag="ava", name="ava0"),
                       pav.tile([P, SQ], f32, tag="avb", name="avb0")),
                   1: (pav1.tile([P, SQ], f32, tag="ava", name="ava1"),
                       pav1.tile([P, SQ], f32, tag="avb", name="avb1"))}

            def pump(s, budget):
                # consume: AV for tiles whose v projection exists (t <= 2s+1)
                for j in (0, 1):
                    while av_next[j] < sc_next[j] and av_next[j] <= 2 * s + 1:
                        t = av_next[j]
                        for half in (0, 1):
                            pp = pp_store.pop((j, t, half))
                            emit_av(avs[j][half], j, t, 1, half, pp)
                        av_next[j] += 1
                # produce: scores for tiles whose kT chunk exists (t <= 4s+3)
                n = 0
                while n < budget:
                    cands = [j for j in (0, 1)
                             if sc_next[j] < NKT and sc_next[j] <= 4 * s + 3
                             and s >= j]
                    if not cands:
                        break
                    j = min(cands, key=lambda jj: sc_next[jj])
                    t = sc_next[j]
                    for half in (0, 1):
                        pp_store[(j, t, half)] = emit_scores(psc0, "sc", j, t, 1, half)
                    sc_next[j] += 1
                    n += 1

            BUDGET = [4, 5, 5, 4, 4, 4, 3, 3]
            for s in range(8):
                jt, ns = s // 4, s % 4
                # v projection tiles 2s, 2s+1 (natural [sk, feat] layout)
                for t in (2 * s, 2 * s + 1):
                    ps = pps.tile([P, KVH * HD], f32, tag="pa", name=f"psv{t}")
                    for kc in range(KC):
                        nc.tensor.matmul(
                            ps, lhsT=x_sb[:, kc, t * P:(t + 1) * P],
                            rhs=wv_sb[:, kc, :],
                            start=(kc == 0), stop=(kc == KC - 1))
                    nc.vector.tensor_copy(
                        out=v_sb[:, t, :, 0:HD],
                        in_=ps.rearrange("p (k h) -> p k h", h=HD))
                # k projection chunk: kT[:, jt, ns*SQ:(ns+1)*SQ]
                ps = pps.tile([P, SQ], f32, tag="pa", name=f"psk{s}")
                for kc in range(KC):
                    nc.tensor.matmul(
                        ps, lhsT=wk_sb[:, kc, jt * P:(jt + 1) * P],
                        rhs=x_sb[:, kc, ns * SQ:(ns + 1) * SQ],
                        start=(kc == 0), stop=(kc == KC - 1))
                nc.vector.tensor_copy(out=kT_sb[:, jt, ns * SQ:(ns + 1) * SQ], in_=ps)
                # q projection for pair s (scale folded into wq on host)
                ps = pps.tile([P, SQ], f32, tag="pa", name=f"psq{s}")
                for kc in range(KC):
                    nc.tensor.matmul(
                        ps, lhsT=wq_sb[:, s, kc, :], rhs=x_sb[:, kc, 0:SQ],
                        start=(kc == 0), stop=(kc == KC - 1))
                nc.vector.tensor_copy(out=qT_sb[:, s, :], in_=ps)
                pump(s, BUDGET[s])
            pump(8, 0)  # drain remaining AV work for pairs 0,1
            assert not pp_store and av_next == {0: NKT, 1: NKT}
            # bridge: score pair 2's first tiles in the still-live psc0 pool
            # so ACT keeps flowing while psc1's banks wait on pool release
            for t in (0, 1):
                for half in (0, 1):
                    pp_store[(2, t, half)] = emit_scores(psc0, "sc", 2, t, 1, half)
            pend[0] = emit_rowsum(0, *avs[0])
            pend[1] = emit_rowsum(1, *avs[1])

        # ---------------- phase B: pairs 2-7, software pipelined ----------------
        # Flat (pair, group) work list, scores emitted two items ahead so the
        # ACT exp pipeline never drains at pair boundaries. Pair 2's tiles
        # 0-1 were already scored into psc0 at the end of phase A.
        BGROUPS = [(0, 3), (3, 3), (6, 3), (9, 3), (12, 3), (15, 1)]
        seq = [(2, t0, ng) for (t0, ng) in
               [(2, 1), (3, 3), (6, 3), (9, 3), (12, 3), (15, 1)]] + \
              [(j, t0, ng) for j in range(3, 8) for (t0, ng) in BGROUPS]
        with tc.tile_pool(name="psc1", bufs=1, space="PSUM") as psc1:
            def b_scores(j, t0, ng):
                return (emit_scores(psc1, "sca", j, t0, ng, 0),
                        emit_scores(psc1, "scb", j, t0, ng, 1))

            avt = {}
            prev_j = None
            pps_q = {i: b_scores(*seq[i]) for i in range(2)}
            for i, (j, t0, ng) in enumerate(seq):
                if j != prev_j:
                    prev_j = j
                    if j - 2 in pend:
                        emit_attnT(j - 2, pend.pop(j - 2))
                    avt[j] = (pav.tile([P, SQ], f32, tag="ava", name=f"ava{j}"),
                              pav.tile([P, SQ], f32, tag="avb", name=f"avb{j}"))
                    if j == 2:
                        for tt in (0, 1):
                            for half in (0, 1):
                                emit_av(avt[2][half], 2, tt, 1, half,
                                        pp_store.pop((2, tt, half)))
                if i + 2 < len(seq):
                    pps_q[i + 2] = b_scores(*seq[i + 2])
                pp0, pp1 = pps_q.pop(i)
                emit_av(avt[j][0], j, t0, ng, 0, pp0)
                emit_av(avt[j][1], j, t0, ng, 1, pp1)
                if t0 + ng == NKT:
                    pend[j] = emit_rowsum(j, *avt.pop(j))
            for j in (6, 7):
                emit_attnT(j, pend.pop(j))
        pav.release()

        # ---------------- phase C: output projection ----------------
        # j=0..6 accumulate into 4 chunk psums first (these only need pairs
        # 0-6, so they fill the pair-7 rowsum latency), then j=7 + stores,
        # then the remaining 4 chunks. pop reuses the score-pool banks, which
        # free as soon as pair 7's last exp is read.
        with tc.tile_pool(name="pop", bufs=4, space="PSUM") as pop, \
             tc.tile_pool(name="osb", bufs=8) as osb:
            def c_chunk_mms(po, ch, js):
                st, nt2 = ch // 2, ch % 2
                for j in js:
                    nc.tensor.matmul(
                        po, lhsT=attnT_sb[:, j, st * P:(st + 1) * P],
                        rhs=wo_sb[:, j, nt2 * SQ:(nt2 + 1) * SQ],
                        start=(j == 0), stop=(j == 7))

            def c_chunk_out(po, ch):
                st, nt2 = ch // 2, ch % 2
                ob = osb.tile([P, SQ], f32, tag="ob", name=f"ob{ch}")
                nc.vector.tensor_copy(out=ob, in_=po)
                nc.sync.dma_start(out=out_r[st, nt2], in_=ob)

            pos = {}
            for ch in range(4):
                pos[ch] = pop.tile([P, SQ], f32, tag="po", name=f"po{ch}")
                c_chunk_mms(pos[ch], ch, range(7))
            for ch in range(4):
                c_chunk_mms(pos[ch], ch, [7])
                c_chunk_out(pos[ch], ch)
            for ch in range(4, 8):
                po = pop.tile([P, SQ], f32, tag="po", name=f"po{ch}")
                c_chunk_mms(po, ch, range(8))
                c_chunk_out(po, ch)
    persist.release()


def _build():
    if "nc" in _CACHE:
        return _CACHE["nc"]
    nc = bacc.Bacc("TRN2", target_bir_lowering=False, debug=False)
    xT = nc.dram_tensor("xT", [D, S], bf16, kind="ExternalInput").ap()
    wqT = nc.dram_tensor("wqT", [8, D, P], bf16, kind="ExternalInput").ap()
    wkT = nc.dram_tensor("wkT", [D, KVH * HD], bf16, kind="ExternalInput").ap()
    wvT = nc.dram_tensor("wvT", [D, KVH * HD], bf16, kind="ExternalInput").ap()
    woT = nc.dram_tensor("woT", [H * HD, D], bf16, kind="ExternalInput").ap()
    emT = nc.dram_tensor("emT", [2, S, SQ], bf16, kind="ExternalInput").ap()
    out = nc.dram_tensor("out", [SQ, D], f32, kind="ExternalOutput").ap()
    with tile.TileContext(nc) as tc:
        _body(tc, xT, wqT, wkT, wvT, woT, emT, out)
    nc.compile()
    _CACHE["nc"] = nc
    return nc


def _host_prep(hidden_states, full_mask, tag_mask, wq, wk, wv, wo):
    # pair-ordered feature permutation for wq columns / wo.T rows
    perm = np.concatenate([np.r_[a * HD:(a + 1) * HD, b * HD:(b + 1) * HD]
                           for a, b in PAIRS])
    import ml_dtypes
    bf = ml_dtypes.bfloat16
    wqTf = np.ascontiguousarray(wq.T[:, perm] * 0.125)             # [D, 1024]
    # [j, p, kc, f] layout so each per-pair chunk DMA reads 2KB/partition runs
    wqT = np.ascontiguousarray(
        wqTf.reshape(KC, P, 8, P).transpose(2, 1, 0, 3)).astype(bf)  # [8,128,8,128]
    wqT = np.ascontiguousarray(wqT.reshape(8, D, P))
    wkT = np.ascontiguousarray(wk.T).astype(bf)                    # [D, 256]
    wvT = np.ascontiguousarray(wv.T).astype(bf)                    # [D, 256]
    woT = np.ascontiguousarray(wo.T[perm, :]).astype(bf)           # [1024, D]
    # exp(mask) transposed to [sk, sq], rolled per core
    emasks = [np.exp(full_mask[b, 0].T) for b in range(B)] + \
             [np.exp(tag_mask[b, 0].T) for b in range(B)]
    xTs = [np.ascontiguousarray(hidden_states[b].T) for b in range(B)]
    in_maps = []
    for c in range(NCORES):
        b, q0 = c // 4, (c % 4) * SQ
        xT_c = np.roll(xTs[b], -q0, axis=1).astype(bf)
        fmT = np.roll(emasks[b][:, q0:q0 + SQ], -q0, axis=0)
        tgT = np.roll(emasks[2 + b][:, q0:q0 + SQ], -q0, axis=0)
        emT_c = np.ascontiguousarray(np.stack([fmT, tgT])).astype(bf)
        in_maps.append({"xT": np.ascontiguousarray(xT_c), "wqT": wqT, "wkT": wkT,
                        "wvT": wvT, "woT": woT, "emT": emT_c})
    return in_maps


def kernel(hidden_states, full_mask, tag_mask, wq, wk, wv, wo, _trace=False):
    args = [np.asarray(a, np.float32) for a in
            (hidden_states, full_mask, tag_mask, wq, wk, wv, wo)]
    nc = _build()
    in_maps = _host_prep(*args)
    try:
        res = run_bass_kernel_spmd(nc, in_maps, core_ids=list(range(NCORES)),
                                   trace=_trace)
    except ModuleNotFoundError:
        res = run_bass_kernel_spmd(nc, in_maps, core_ids=list(range(NCORES)))
    _CACHE["last_results"] = res
    full = np.empty((B, S, D), np.float32)
    for c in range(NCORES):
        b, q0 = c // 4, (c % 4) * SQ
        full[b, q0:q0 + SQ, :] = res.results[c]["out"]
    return full


# revision 84
# speedup vs baseline: 1.0630x; 1.0630x over previous
"""Trainium2 Bass kernel for nn_JanusModel (sparse_attention, GQA, two mask groups).

Sharding (head-group): core c = (batch c//4, head-group (c%4)//2, q-half c%2).
Each core computes its group's 8 heads (4 real pairs x 2 q-chunks = 8 virtual
pairs of 512 q rows) over the full 2048 keys; outputs are partial over heads
and summed pairwise on the host. No collectives. Heavy operands laid out on host (transposes/permutes, exp(mask),
1/sqrt(hd) folded into wq, bf16 casts).

On-device math per core (all bf16 operands, fp32 PSUM):
  qT/kT/v projections, scoresT = K @ qT (row-tiled head pairs, [sk, sq] layout),
  P = exp(scores) * expm (ACT exp + DVE bf16 mul), AV matmuls with a ones-column
  appended to V so row 64 of each AV psum accumulates the softmax denominator,
  reciprocal + DMA broadcast, output projection accumulated over head pairs.

Pipelining: pair 0's whole attention is interleaved into the projection loop so
the ACT engine starts exp work ~10us in; pairs 1-7 run a software-pipelined
PE emission (scores of group g+1 issued before AV of group g); attnT division
for pair j is deferred to pair j+2 so the rowsum DMA roundtrip never stalls PE.
"""

import os
import sys

import numpy as np

for _p in ("/opt/trn_rl_repo",):
    if os.path.isdir(_p) and _p not in sys.path:
        sys.path.insert(0, _p)

import concourse.bass as bass
import concourse.tile as tile
from concourse import bacc, mybir
from concourse.bass_utils import run_bass_kernel_spmd

B, S, D = 2, 2048, 1024
H, KVH, HD = 16, 4, 64
NCORES = 8
SQ = S // 4          # 512 query rows per core
P = 128
NKT = S // P         # 16 key tiles
KC = D // P          # 8 contraction chunks for projections
NG = 8               # 2-tile k groups per half

# Head pairs: (a, b) share a kT tile; a uses kv head 2*(j//4), b uses +1.
PAIRS = [(0, 4), (1, 5), (2, 6), (3, 7), (8, 12), (9, 13), (10, 14), (11, 15)]

f32 = mybir.dt.float32
bf16 = mybir.dt.bfloat16
EXP = mybir.ActivationFunctionType.Exp

_CACHE = {}


def _body(tc, xT, wqT, wkT, wvT, woT, emT, out):
    nc = tc.nc
    rs_dram = nc.dram_tensor("rs_scratch", [8, 2, SQ], bf16).ap()
    xT_r = xT.rearrange("(kc p) (c s) -> c p kc s", p=P, s=SQ)   # [4,128,8,512]
    wqT_r = wqT.rearrange("j (p kc) f -> j p kc f", p=P)         # [4,128,8,128]
    wkT_r = wkT.rearrange("(kc p) f -> p kc f", p=P)             # [128,8,256]
    wvT_r = wvT.rearrange("(kc p) f -> p kc f", p=P)             # [128,8,256]
    woT_r = woT.rearrange("(j p) d -> p j d", p=P)               # [128,4,1024]
    emT_r = emT.rearrange("m (t p) q -> m p t q", p=P)           # [2,128,16,512]
    out_r = out.rearrange("(t p) (n q) -> t n p q", p=P, q=SQ)   # [8,2,128,512]

    persist = tc.alloc_tile_pool(name="persist", bufs=1)
    qT_sb = persist.tile([P, 8, SQ], bf16, name="qT_sb")      # pair j: a 0:64, b 64:128
    kT_sb = persist.tile([P, 1, S], bf16, name="kT_sb")       # jt: kv 2jt 0:64, 2jt+1 64:128
    v_sb = persist.tile([P, NKT, 2, HD + 1], bf16, name="v_sb")  # col HD = ones
    em_sb = persist.tile([P, 2, NKT, SQ], bf16, name="em_sb")
    attnT_sb = persist.tile([P, 8, SQ], bf16, name="attnT_sb")
    wo_sb = persist.tile([P, 4, D], bf16, name="wo_sb")
    warm = persist.tile([1, 2], bf16, name="warm")

    # ---- all input DMAs, ordered for earliest compute start (one SP queue) ----
    pav = tc.alloc_tile_pool(name="pav", bufs=1, space="PSUM")
    with tc.tile_pool(name="xw", bufs=1) as xw, \
         tc.tile_pool(name="prp", bufs=3) as prp, \
         tc.tile_pool(name="ppp", bufs=6) as ppp, \
         tc.tile_pool(name="avsb", bufs=3) as avsbp, \
         tc.tile_pool(name="small", bufs=2) as small:
        x_sb = xw.tile([P, KC, S], bf16, name="x_sb")
        wq_sb = xw.tile([P, 4, KC, P], bf16, name="wq_sb")
        wk_sb = xw.tile([P, KC, 2 * HD], bf16, name="wk_sb")
        wv_sb = xw.tile([P, KC, 2 * HD], bf16, name="wv_sb")
        nc.sync.dma_start(out=wv_sb, in_=wvT_r)
        nc.sync.dma_start(out=x_sb[:, :, 0:P], in_=xT_r[0, :, :, 0:P])
        nc.sync.dma_start(out=x_sb[:, :, P:SQ], in_=xT_r[0, :, :, P:SQ])
        nc.sync.dma_start(out=wk_sb, in_=wkT_r)
        nc.sync.dma_start(out=wq_sb[:, 0], in_=wqT_r[0])
        nc.sync.dma_start(out=em_sb[:, 0, 0:8, :], in_=emT_r[0, :, 0:8, :])
        nc.sync.dma_start(out=x_sb[:, :, SQ:2 * SQ], in_=xT_r[1])
        nc.sync.dma_start(out=wq_sb[:, 1], in_=wqT_r[1])
        nc.sync.dma_start(out=em_sb[:, 0, 8:16, :], in_=emT_r[0, :, 8:16, :])
        nc.sync.dma_start(out=x_sb[:, :, 2 * SQ:3 * SQ], in_=xT_r[2])
        for j in (2, 3):
            nc.sync.dma_start(out=wq_sb[:, j], in_=wqT_r[j])  # all 4 chunks
        nc.sync.dma_start(out=x_sb[:, :, 3 * SQ:4 * SQ], in_=xT_r[3])
        nc.sync.dma_start(out=em_sb[:, 1], in_=emT_r[1])
        nc.sync.dma_start(out=wo_sb, in_=woT_r)

        # ACT exp table warm-up + ones column of v_sb
        nc.vector.memset(warm, 0.0)
        nc.scalar.activation(out=warm, in_=warm, func=EXP)
        nc.vector.memset(v_sb[:, :, :, HD:HD + 1], 1.0)

        # ---------- emission helpers ----------
        def emit_scores(pool, tag, j, t0, ntg, half):
            """ntg score matmuls for k-tiles t0.. -> exp -> mask mul; returns pp."""
            jt = 0
            r0, r1 = 64 * half, 64 * half + 64
            sc = pool.tile([P, ntg, SQ], f32, tag=tag, name=f"sc{j}_{t0}_{half}")
            for i in range(ntg):
                t = t0 + i
                nc.tensor.matmul(
                    sc[:, i, :], lhsT=kT_sb[r0:r1, jt, t * P:(t + 1) * P],
                    rhs=qT_sb[r0:r1, j, :], start=True, stop=True)
            pr = prp.tile([P, ntg, SQ], bf16, tag="pr", name=f"pr{j}_{t0}_{half}")
            nc.scalar.activation(out=pr, in_=sc, func=EXP)
            pp = ppp.tile([P, ntg, SQ], bf16, tag="pp", name=f"pp{j}_{t0}_{half}")
            nc.vector.tensor_mul(pp, pr, em_sb[:, j % 2, t0:t0 + ntg, :])
            return pp

        def emit_av(av, j, t0, ntg, half, pp):
            kv = half
            for i in range(ntg):
                t = t0 + i
                nc.tensor.matmul(
                    av[0:HD + 1, :], lhsT=v_sb[:, t, kv, :], rhs=pp[:, i, :],
                    start=(t == 0), stop=(t == NKT - 1))

        def emit_rowsum(j, av_a, av_b):
            """Copy AV numerators+rowsums psum->sbuf bf16 (the only readers of
            the av psum banks, so they free ~1.3us after the pair ends), then
            reciprocal from the copy, DMA roundtrip broadcast in bf16. The
            attnT division is deferred to pair j+2. Returns (bc, avsb)."""
            avsb = avsbp.tile([P, 2, SQ], bf16, tag="av", name=f"avsb{j}")
            nc.vector.tensor_copy(out=avsb[0:HD + 1, 0, :], in_=av_a[0:HD + 1, :])
            nc.vector.tensor_copy(out=avsb[0:HD + 1, 1, :], in_=av_b[0:HD + 1, :])
            rr = small.tile([P, 2, SQ], bf16, tag="rr", name=f"rr{j}")
            with nc.allow_low_precision(reason="bf16 softmax denominators"):
                nc.vector.reciprocal(out=rr[HD:HD + 1, 0, :],
                                     in_=avsb[HD:HD + 1, 0, :])
                nc.vector.reciprocal(out=rr[HD:HD + 1, 1, :],
                                     in_=avsb[HD:HD + 1, 1, :])
            nc.sync.dma_start(out=rs_dram[j], in_=rr[HD:HD + 1, :, :])
            bc = small.tile([P, 2, SQ], bf16, tag="bc", name=f"bc{j}")
            for half in range(2):
                row = rs_dram[j, half, :]
                bcast = bass.AP(tensor=row.tensor, offset=row.offset,
                                ap=[[0, 64]] + list(row.ap))
                nc.sync.dma_start(out=bc[0:64, half, :], in_=bcast)
            return bc, avsb

        def emit_attnT(j, pend):
            bc, avsb = pend
            nc.vector.tensor_mul(attnT_sb[0:64, j, :], avsb[0:HD, 0, :],
                                 bc[0:64, 0, :])
            nc.vector.tensor_mul(attnT_sb[64:128, j, :], avsb[0:HD, 1, :],
                                 bc[0:64, 1, :])

        # ---------------- phase A + pairs 0,1 interleaved ----------------
        # Pairs 0 and 1 share one score psum buffer (exps ping-pong through
        # it), keeping ACT busy while the PE runs projection chains. AV
        # consumption lags scores until the v tiles exist; leftover pp tiles
        # are retained in the ppp pool.
        pend = {}
        pp_store = {}
        sc_next = {0: 0, 1: 0}   # next k-TILE (1-tile groups during phase A)
        av_next = {0: 0, 1: 0}
        with tc.tile_pool(name="pps", bufs=2, space="PSUM") as pps, \
             tc.tile_pool(name="psc0", bufs=2, space="PSUM") as psc0, \
             tc.tile_pool(name="pav1", bufs=1, space="PSUM") as pav1:
            avs = {0: (pav.tile([P, SQ], f32, tag="ava", name="ava0"),
                       pav.tile([P, SQ], f32, tag="avb", name="avb0")),
                   1: (pav1.tile([P, SQ], f32, tag="ava", name="ava1"),
                       pav1.tile([P, SQ], f32, tag="avb", name="avb1"))}

            def pump(s, budget):
                # consume: AV for tiles whose v projection exists (t <= 2s+1)
                for j in (0, 1):
                    while av_next[j] < sc_next[j] and av_next[j] <= 2 * s + 1:
                        t = av_next[j]
                        for half in (0, 1):
                            pp = pp_store.pop((j, t, half))
                            emit_av(avs[j][half], j, t, 1, half, pp)
                        av_next[j] += 1
                # produce: scores for tiles whose kT chunk exists (t <= 4s+3)
                n = 0
                while n < budget:
                    cands = [j for j in (0, 1)
                             if sc_next[j] < NKT and sc_next[j] <= 4 * s + 3
                             and s >= j]
                    if not cands:
                        break
                    j = min(cands, key=lambda jj: sc_next[jj])
                    t = sc_next[j]
                    for half in (0, 1):
                        pp_store[(j, t, half)] = emit_scores(psc0, "sc", j, t, 1, half)
                    sc_next[j] += 1
                    n += 1

            BUDGET = [4, 5, 5, 4, 4, 4, 3, 3]
            for s in range(8):
                jt, ns = 0, s
                # v projection tiles 2s, 2s+1 (natural [sk, feat] layout)
                for t in (2 * s, 2 * s + 1):
                    ps = pps.tile([P, 2 * HD], f32, tag="pa", name=f"psv{t}")
                    for kc in range(KC):
                        nc.tensor.matmul(
                            ps, lhsT=x_sb[:, kc, t * P:(t + 1) * P],
                            rhs=wv_sb[:, kc, :],
                            start=(kc == 0), stop=(kc == KC - 1))
                    nc.vector.tensor_copy(
                        out=v_sb[:, t, :, 0:HD],
                        in_=ps.rearrange("p (k h) -> p k h", h=HD))
                if s < 4:
                    # k projection chunk: kT[:, 0, s*SQ:(s+1)*SQ]
                    ps = pps.tile([P, SQ], f32, tag="pa", name=f"psk{s}")
                    for kc in range(KC):
                        nc.tensor.matmul(
                            ps, lhsT=wk_sb[:, kc, :],
                            rhs=x_sb[:, kc, ns * SQ:(ns + 1) * SQ],
                            start=(kc == 0), stop=(kc == KC - 1))
                    nc.vector.tensor_copy(
                        out=kT_sb[:, 0, ns * SQ:(ns + 1) * SQ], in_=ps)
                # q projection for pair s (scale folded into wq on host)
                ps = pps.tile([P, SQ], f32, tag="pa", name=f"psq{s}")
                for kc in range(KC):
                    nc.tensor.matmul(
                        ps, lhsT=wq_sb[:, s // 2, kc, :],
                        rhs=x_sb[:, kc, (s % 2) * SQ:(s % 2 + 1) * SQ],
                        start=(kc == 0), stop=(kc == KC - 1))
                nc.vector.tensor_copy(out=qT_sb[:, s, :], in_=ps)
                pump(s, BUDGET[s])
            pump(8, 0)  # drain remaining AV work for pairs 0,1
            assert not pp_store and av_next == {0: NKT, 1: NKT}
            # bridge: score pair 2's first tiles in the still-live psc0 pool
            # so ACT keeps flowing while psc1's banks wait on pool release
            for t in (0, 1):
                for half in (0, 1):
                    pp_store[(2, t, half)] = emit_scores(psc0, "sc", 2, t, 1, half)
            pend[0] = emit_rowsum(0, *avs[0])
            pend[1] = emit_rowsum(1, *avs[1])

        # ---------------- phase B: pairs 2-7, software pipelined ----------------
        # Flat (pair, group) work list, scores emitted two items ahead so the
        # ACT exp pipeline never drains at pair boundaries. Pair 2's tiles
        # 0-1 were already scored into psc0 at the end of phase A.
        BGROUPS = [(0, 3), (3, 3), (6, 3), (9, 3), (12, 3), (15, 1)]
        seq = [(2, t0, ng) for (t0, ng) in
               [(2, 1), (3, 3), (6, 3), (9, 3), (12, 3), (15, 1)]] + \
              [(j, t0, ng) for j in range(3, 8) for (t0, ng) in BGROUPS]
        with tc.tile_pool(name="psc1", bufs=1, space="PSUM") as psc1:
            def b_scores(j, t0, ng):
                return (emit_scores(psc1, "sca", j, t0, ng, 0),
                        emit_scores(psc1, "scb", j, t0, ng, 1))

            avt = {}
            prev_j = None
            pps_q = {i: b_scores(*seq[i]) for i in range(2)}
            for i, (j, t0, ng) in enumerate(seq):
                if j != prev_j:
                    prev_j = j
                    if j - 2 in pend:
                        emit_attnT(j - 2, pend.pop(j - 2))
                    avt[j] = (pav.tile([P, SQ], f32, tag="ava", name=f"ava{j}"),
                              pav.tile([P, SQ], f32, tag="avb", name=f"avb{j}"))
                    if j == 2:
                        for tt in (0, 1):
                            for half in (0, 1):
                                emit_av(avt[2][half], 2, tt, 1, half,
                                        pp_store.pop((2, tt, half)))
                if i + 2 < len(seq):
                    pps_q[i + 2] = b_scores(*seq[i + 2])
                pp0, pp1 = pps_q.pop(i)
                emit_av(avt[j][0], j, t0, ng, 0, pp0)
                emit_av(avt[j][1], j, t0, ng, 1, pp1)
                if t0 + ng == NKT:
                    pend[j] = emit_rowsum(j, *avt.pop(j))
            for j in (6, 7):
                emit_attnT(j, pend.pop(j))
        pav.release()

        # ---------------- phase C: output projection ----------------
        # j=0..6 accumulate into 4 chunk psums first (these only need pairs
        # 0-6, so they fill the pair-7 rowsum latency), then j=7 + stores,
        # then the remaining 4 chunks. pop reuses the score-pool banks, which
        # free as soon as pair 7's last exp is read.
        with tc.tile_pool(name="pop", bufs=4, space="PSUM") as pop, \
             tc.tile_pool(name="osb", bufs=8) as osb:
            def c_chunk_mms(po, ch, js):
                st, nt2 = ch // 2, ch % 2
                for j in js:  # j = real pair 0..3
                    nc.tensor.matmul(
                        po,
                        lhsT=attnT_sb[:, 2 * j + st // 4, (st % 4) * P:
                                      (st % 4 + 1) * P],
                        rhs=wo_sb[:, j, nt2 * SQ:(nt2 + 1) * SQ],
                        start=(j == 0), stop=(j == 3))

            def c_chunk_out(po, ch):
                st, nt2 = ch // 2, ch % 2
                ob = osb.tile([P, SQ], bf16, tag="ob", name=f"ob{ch}")
                nc.vector.tensor_copy(out=ob, in_=po)
                nc.sync.dma_start(out=out_r[st, nt2], in_=ob)

            pos = {}
            for ch in range(4):
                pos[ch] = pop.tile([P, SQ], f32, tag="po", name=f"po{ch}")
                c_chunk_mms(pos[ch], ch, range(3))
            for ch in range(4):
                c_chunk_mms(pos[ch], ch, [3])
                c_chunk_out(pos[ch], ch)
            for ch in range(4, 16):
                po = pop.tile([P, SQ], f32, tag="po", name=f"po{ch}")
                c_chunk_mms(po, ch, range(4))
                c_chunk_out(po, ch)
    persist.release()


def _build():
    if "nc" in _CACHE:
        return _CACHE["nc"]
    nc = bacc.Bacc("TRN2", target_bir_lowering=False, debug=False)
    xT = nc.dram_tensor("xT", [D, S], bf16, kind="ExternalInput").ap()
    wqT = nc.dram_tensor("wqT", [4, D, P], bf16, kind="ExternalInput").ap()
    wkT = nc.dram_tensor("wkT", [D, 2 * HD], bf16, kind="ExternalInput").ap()
    wvT = nc.dram_tensor("wvT", [D, 2 * HD], bf16, kind="ExternalInput").ap()
    woT = nc.dram_tensor("woT", [8 * HD, D], bf16, kind="ExternalInput").ap()
    emT = nc.dram_tensor("emT", [2, S, SQ], bf16, kind="ExternalInput").ap()  # [qchunk, k, q]
    out = nc.dram_tensor("out", [2 * SQ, D], bf16, kind="ExternalOutput").ap()
    with tile.TileContext(nc) as tc:
        _body(tc, xT, wqT, wkT, wvT, woT, emT, out)
    nc.compile()
    _CACHE["nc"] = nc
    return nc


def _host_prep(hidden_states, full_mask, tag_mask, wq, wk, wv, wo):
    import ml_dtypes
    bf = ml_dtypes.bfloat16
    xTs = [np.ascontiguousarray(hidden_states[b].T) for b in range(B)]
    emasks = [np.exp(full_mask[b, 0].T) for b in range(B)], \
             [np.exp(tag_mask[b, 0].T) for b in range(B)]
    # per-group weight slices (pair-ordered feature permutation within group)
    wqTs, wkTs, wvTs, woTs = [], [], [], []
    for grp in range(2):
        pairs_g = PAIRS[4 * grp:4 * grp + 4]
        perm = np.concatenate([np.r_[a * HD:(a + 1) * HD, b2 * HD:(b2 + 1) * HD]
                               for a, b2 in pairs_g])
        wqTf = np.ascontiguousarray(wq.T[:, perm] * 0.125)        # [D, 512]
        wqT = np.ascontiguousarray(
            wqTf.reshape(KC, P, 4, P).transpose(2, 1, 0, 3)).astype(bf)
        wqTs.append(np.ascontiguousarray(wqT.reshape(4, D, P)))
        wkTs.append(np.ascontiguousarray(
            wk.T[:, grp * 2 * HD:(grp + 1) * 2 * HD]).astype(bf))  # [D, 128]
        wvTs.append(np.ascontiguousarray(
            wv.T[:, grp * 2 * HD:(grp + 1) * 2 * HD]).astype(bf))
        woTs.append(np.ascontiguousarray(wo.T[perm, :]).astype(bf))  # [512, D]
    in_maps = []
    for c in range(NCORES):
        b, grp, qh = c // 4, (c % 4) // 2, c % 2
        q0 = qh * 2 * SQ
        xT_c = np.roll(xTs[b], -q0, axis=1).astype(bf)
        em = np.roll(emasks[grp][b][:, q0:q0 + 2 * SQ], -q0, axis=0)
        emT_c = np.ascontiguousarray(
            np.stack([em[:, 0:SQ], em[:, SQ:2 * SQ]])).astype(bf)
        in_maps.append({"xT": np.ascontiguousarray(xT_c), "wqT": wqTs[grp],
                        "wkT": wkTs[grp], "wvT": wvTs[grp],
                        "woT": woTs[grp], "emT": emT_c})
    return in_maps


def kernel(hidden_states, full_mask, tag_mask, wq, wk, wv, wo, _trace=False):
    args = [np.asarray(a, np.float32) for a in
            (hidden_states, full_mask, tag_mask, wq, wk, wv, wo)]
    nc = _build()
    in_maps = _host_prep(*args)
    try:
        res = run_bass_kernel_spmd(nc, in_maps, core_ids=list(range(NCORES)),
                                   trace=_trace)
    except ModuleNotFoundError:
        res = run_bass_kernel_spmd(nc, in_maps, core_ids=list(range(NCORES)))
    _CACHE["last_results"] = res
    full = np.empty((B, S, D), np.float32)
    for b in range(B):
        for qh in range(2):
            q0 = qh * 2 * SQ
            full[b, q0:q0 + 2 * SQ, :] = (
                res.results[b * 4 + qh]["out"].astype(np.float32) +
                res.results[b * 4 + 2 + qh]["out"].astype(np.float32))
    return full
